# revision 7
# baseline (speedup 1.0000x reference)
"""Trainium2 Bass kernel for nn_Controller (4-layer LSTM-like controller + heads).

Sharding: tensor-parallel over the hidden dim H=1024 -> 128 h-columns per core
for the stacked gate GEMMs, with an AllGather of each layer's h to rebuild the
full hidden state for the next layer's recurrence and the output heads. Heads
are sharded by output column (64 of 512 y-cols + 32 of 256 E-cols per core).

The batch (512) is processed in two halves of 256 so each layer's AllGather
(bf16 [128,256] per core -> Mesh algorithm, ~6.5us) pipelines with the other
half's compute; the collective stream is the critical resource. All matmuls
run in bf16 with fp32 PSUM accumulation; the cell state c and the gate
nonlinearities stay in fp32.
"""

import numpy as np
import ml_dtypes

import concourse.bass as bass
import concourse.mybir as mybir
import concourse.tile as tile
from concourse import bacc
from concourse.bass_utils import run_bass_kernel_spmd

BF16 = ml_dtypes.bfloat16

# Problem constants (hardcoded; kernel.py must be self-contained).
P_X, P_H, P_L, P_VT, P_ET, B = 512, 1024, 4, 512, 256, 512
IN_DIM = P_X + 2 * P_H  # 2560
NCORES = 8
HS = P_H // NCORES       # 128 h-columns per core
YW = P_VT // NCORES      # 64 y output cols per core
EW = P_ET // NCORES      # 32 E output cols per core
MW = YW + EW             # 96 head output cols per core
G = 4                    # gates (i, f, o, s)
NK_X = P_X // 128        # 4 K-tiles from x
NK_H = P_H // 128        # 8 K-tiles from h
NKT = NK_X + 2 * NK_H    # 20 K-tiles total per layer
NPRE = NK_X + NK_H       # 12 K-tiles with no cross-core dependency
NK_HEAD = (P_L * P_H) // 128  # 32 K-tiles for the heads
HB = B // 2              # batch half

_CACHE = {}


def _build_nc():
    dt = mybir.dt
    AF = mybir.ActivationFunctionType

    nc = bacc.Bacc("TRN2", target_bir_lowering=False, debug=False,
                   num_devices=NCORES)

    wg_d = nc.dram_tensor("wg", [P_L, 128, NKT * G * HS], dt.bfloat16,
                          kind="ExternalInput")
    xt_d = nc.dram_tensor("xt", [128, NK_X * B], dt.bfloat16,
                          kind="ExternalInput")
    hpt_d = nc.dram_tensor("hpt", [P_L, 128, NK_H * B], dt.bfloat16,
                           kind="ExternalInput")
    cpt_d = nc.dram_tensor("cpt", [P_L, 128, B], dt.float32,
                           kind="ExternalInput")
    bg_d = nc.dram_tensor("bg", [128, P_L * G], dt.float32,
                          kind="ExternalInput")
    wyE_d = nc.dram_tensor("wyE", [128, NK_HEAD * MW], dt.bfloat16,
                           kind="ExternalInput")
    byE_d = nc.dram_tensor("byE", [MW, 1], dt.float32, kind="ExternalInput")
    out_d = nc.dram_tensor("oyE", [MW, B], dt.float32, kind="ExternalOutput")

    rg = [list(range(NCORES))]
    gate_order = [0, 3, 1, 2]  # i, s first (product needed first), then f, o

    with tile.TileContext(nc) as tc:
        with (
            tc.tile_pool(name="wpool", bufs=1) as wpool,
            tc.tile_pool(name="wgpool", bufs=2) as wgpool,
            tc.tile_pool(name="apool", bufs=2) as apool,
            tc.tile_pool(name="psum", bufs=1, space="PSUM") as psum,
            tc.tile_pool(name="dram", bufs=2, space="DRAM") as dram,
        ):
            # Layer-0-critical loads first, in consumption order.
            xt_sb = wpool.tile([128, NK_X, B], dt.bfloat16, name="xt_sb")
            nc.sync.dma_start(xt_sb[:], xt_d.ap().rearrange(
                "p (k n) -> p k n", k=NK_X))
            wg_sb = [None] * P_L
            wg_sb[0] = wgpool.tile([128, NKT, G * HS], dt.bfloat16,
                                   name="wg0", tag="wg")
            wgv0 = wg_d[0].rearrange("p (k m) -> p k m", k=NKT)
            for c0, c1 in ((0, 4), (4, 8), (8, 12)):
                nc.sync.dma_start(wg_sb[0][:, c0:c1, :], wgv0[:, c0:c1, :])
            hpt_sb = [None] * P_L
            hpt_sb[0] = wpool.tile([128, NK_H, B], dt.bfloat16, name="hpt0",
                                   tag="hpt0")
            nc.sync.dma_start(hpt_sb[0][:], hpt_d[0].rearrange(
                "p (k n) -> p k n", k=NK_H))
            bg_sb = wpool.tile([128, P_L * G], dt.float32, name="bg_sb")
            nc.sync.dma_start(bg_sb[:], bg_d[:])
            cpt_sb = [None] * P_L
            cpt_sb[0] = wpool.tile([128, B], dt.float32, name="cpt0",
                                   tag="cpt0")
            nc.sync.dma_start(cpt_sb[0][:], cpt_d[0])

            # Remaining loads, layer-ordered so they stream just ahead of use.
            for l in range(1, P_L):
                hpt_sb[l] = wpool.tile([128, NK_H, B], dt.bfloat16,
                                       name=f"hpt{l}", tag=f"hpt{l}")
                nc.sync.dma_start(hpt_sb[l][:], hpt_d[l].rearrange(
                    "p (k n) -> p k n", k=NK_H))
                wg_sb[l] = wgpool.tile([128, NKT, G * HS], dt.bfloat16,
                                       name=f"wg{l}", tag="wg")
                wgv = wg_d[l].rearrange("p (k m) -> p k m", k=NKT)
                for c0, c1 in ((0, 5), (5, 10), (10, 15), (15, 20)):
                    nc.sync.dma_start(wg_sb[l][:, c0:c1, :], wgv[:, c0:c1, :])
                cpt_sb[l] = wpool.tile([128, B], dt.float32, name=f"cpt{l}",
                                       tag=f"cpt{l}")
                nc.sync.dma_start(cpt_sb[l][:], cpt_d[l])
            wyE_sb = wpool.tile([128, NK_HEAD, MW], dt.bfloat16, name="wyE_sb")
            nc.sync.dma_start(wyE_sb[:], wyE_d.ap().rearrange(
                "p (k m) -> p k m", k=NK_HEAD))
            byE_sb = wpool.tile([MW, 1], dt.float32, name="byE_sb")
            nc.sync.dma_start(byE_sb[:], byE_d[:])

            hfull = [None] * P_L  # gathered full h.T per layer (bf16)

            for l in range(P_L):
                nkt = NPRE if l == 0 else NKT

                def rhs(kt, hs):
                    if kt < NK_X:
                        return xt_sb[:, kt, hs]
                    if kt < NPRE:
                        return hpt_sb[l][:, kt - NK_X, hs]
                    return hfull[l - 1][:, kt - NPRE, hs]

                halves = [slice(0, HB), slice(HB, B)]
                ps = {}
                for X in (0, 1):
                    for g in gate_order:
                        ps[(g, X)] = psum.tile(
                            [128, HB], dt.float32, name=f"ps{l}{g}{X}",
                            tag=f"g{g}{X}")
                # Pre phase: contributions from x and h_prev (input-only deps).
                for X in (0, 1):
                    for g in gate_order:
                        for kt in range(NPRE):
                            nc.tensor.matmul(
                                ps[(g, X)][:],
                                wg_sb[l][:, kt, g * HS:(g + 1) * HS],
                                rhs(kt, halves[X]), start=(kt == 0),
                                stop=(kt == nkt - 1))
                # Prev phase per half (waits on that half's AllGather), then
                # activations + AllGather for that half.
                for X in (0, 1):
                    if l > 0:
                        for g in gate_order:
                            for kt in range(NPRE, nkt):
                                nc.tensor.matmul(
                                    ps[(g, X)][:],
                                    wg_sb[l][:, kt, g * HS:(g + 1) * HS],
                                    rhs(kt, halves[X]), start=False,
                                    stop=(kt == nkt - 1))
                    hs = halves[X]
                    bidx = lambda g: l * G + g
                    i_s = apool.tile([128, HB], dt.float32, name="i_s",
                                     tag="i_s")
                    nc.scalar.activation(i_s[:], ps[(0, X)][:], AF.Sigmoid,
                                         bias=bg_sb[:, bidx(0):bidx(0) + 1])
                    s_s = apool.tile([128, HB], dt.float32, name="s_s",
                                     tag="s_s")
                    nc.scalar.activation(s_s[:], ps[(3, X)][:], AF.Tanh,
                                         bias=bg_sb[:, bidx(3):bidx(3) + 1])
                    t1 = apool.tile([128, HB], dt.float32, name="t1", tag="t1")
                    nc.vector.tensor_mul(t1[:], i_s[:], s_s[:])
                    f_s = apool.tile([128, HB], dt.float32, name="f_s",
                                     tag="f_s")
                    nc.scalar.activation(f_s[:], ps[(1, X)][:], AF.Sigmoid,
                                         bias=bg_sb[:, bidx(1):bidx(1) + 1])
                    t2 = apool.tile([128, HB], dt.float32, name="t2", tag="t2")
                    nc.vector.tensor_mul(t2[:], f_s[:], cpt_sb[l][:, hs])
                    c_n = apool.tile([128, HB], dt.float32, name="c_n",
                                     tag="c_n")
                    nc.vector.tensor_add(c_n[:], t1[:], t2[:])
                    tc_n = apool.tile([128, HB], dt.float32, name="tc_n",
                                      tag="tc_n")
                    nc.scalar.activation(tc_n[:], c_n[:], AF.Tanh)
                    o_s = apool.tile([128, HB], dt.float32, name="o_s",
                                     tag="o_s")
                    nc.scalar.activation(o_s[:], ps[(2, X)][:], AF.Sigmoid,
                                         bias=bg_sb[:, bidx(2):bidx(2) + 1])
                    h_bf = apool.tile([128, HB], dt.bfloat16, name="h_bf",
                                      tag="h_bf")
                    nc.vector.tensor_mul(h_bf[:], o_s[:], tc_n[:])

                    ag_in = dram.tile([128, HB], dt.bfloat16,
                                      name=f"agin{l}{X}", tag="ag_in")
                    nc.gpsimd.dma_start(ag_in[:], h_bf[:])
                    ag_out = dram.tile([NCORES * 128, HB], dt.bfloat16,
                                       name=f"agout{l}{X}", tag="ag_out",
                                       addr_space="Shared")
                    nc.gpsimd.collective_compute(
                        "AllGather", mybir.AluOpType.bypass, replica_groups=rg,
                        ins=[ag_in.opt()], outs=[ag_out.opt()])
                    if hfull[l] is None:
                        hfull[l] = wpool.tile([128, NK_H, B], dt.bfloat16,
                                              name=f"hf{l}", tag=f"hf{l}")
                    agv = ag_out.rearrange("(r p) n -> p r n", p=128)
                    nc.sync.dma_start(hfull[l][:, 0:4, hs], agv[:, 0:4, :])
                    nc.sync.dma_start(hfull[l][:, 4:8, hs], agv[:, 4:8, :])

            # Output heads, per batch half: this core's [96, HB] slice of
            # (y | E), contracting over the full 4096-dim hidden state.
            for X in (0, 1):
                hs = slice(X * HB, (X + 1) * HB)
                ps_y = psum.tile([MW, HB], dt.float32, name=f"ps_y{X}",
                                 tag=f"g0{X}")
                for ki in range(NK_HEAD):
                    l, j = divmod(ki, NK_H)
                    nc.tensor.matmul(ps_y[:], wyE_sb[:, ki, :],
                                     hfull[l][:, j, hs],
                                     start=(ki == 0), stop=(ki == NK_HEAD - 1))
                outy = apool.tile([MW, HB], dt.float32, name="outy", tag="outy")
                nc.scalar.activation(outy[:], ps_y[:], AF.Identity,
                                     bias=byE_sb[:, 0:1])
                nc.sync.dma_start(out_d[:, hs], outy[:])

    nc.compile()
    return nc


def _shard_inputs(inputs):
    """Host-side: transpose/convert/shard the full inputs into 8 per-core maps."""
    x = np.ascontiguousarray(inputs["input_x"], dtype=np.float32)
    h_prev = np.ascontiguousarray(inputs["h_prev"], dtype=np.float32)
    c_prev = np.ascontiguousarray(inputs["c_prev"], dtype=np.float32)
    Wg = np.ascontiguousarray(inputs["Wg"], dtype=np.float32)
    bg = np.ascontiguousarray(inputs["bg"], dtype=np.float32)
    W_y = np.ascontiguousarray(inputs["W_y"], dtype=np.float32)
    b_y = np.ascontiguousarray(inputs["b_y"], dtype=np.float32)
    W_E = np.ascontiguousarray(inputs["W_E"], dtype=np.float32)
    b_E = np.ascontiguousarray(inputs["b_E"], dtype=np.float32)

    # Shared across cores.
    xt = x.T.reshape(NK_X, 128, B).transpose(1, 0, 2).reshape(128, NK_X * B)
    xt = xt.astype(BF16)
    hpt = h_prev.transpose(0, 2, 1).reshape(P_L, NK_H, 128, B)
    hpt = hpt.transpose(0, 2, 1, 3).reshape(P_L, 128, NK_H * B).astype(BF16)

    wg_t = Wg.transpose(0, 3, 1, 2)  # [L, IN_DIM, G, H]

    in_maps = []
    for k in range(NCORES):
        wgk = wg_t[:, :, :, k * HS:(k + 1) * HS]          # [L, IN_DIM, G, HS]
        wgk = wgk.reshape(P_L, NKT, 128, G * HS)           # (l, kt, p, m)
        wgk = wgk.transpose(0, 2, 1, 3)                    # (l, p, kt, m)
        wgk = np.ascontiguousarray(wgk).reshape(P_L, 128, NKT * G * HS)
        wgk = wgk.astype(BF16)

        bgk = bg[:, :, k * HS:(k + 1) * HS]                # [L, G, HS]
        bgk = np.ascontiguousarray(bgk.transpose(2, 0, 1)).reshape(
            128, P_L * G).astype(np.float32)

        cptk = np.ascontiguousarray(
            c_prev[:, :, k * HS:(k + 1) * HS].transpose(0, 2, 1)).astype(
            np.float32)                                    # [L, 128, B]

        wy = W_y[k * YW:(k + 1) * YW]                      # [YW, 4096]
        wE = W_E[k * EW:(k + 1) * EW]                      # [EW, 4096]
        wyE = np.concatenate([wy, wE], axis=0).T           # [4096, MW]
        wyE = wyE.reshape(NK_HEAD, 128, MW).transpose(1, 0, 2)
        wyE = np.ascontiguousarray(wyE).reshape(128, NK_HEAD * MW).astype(BF16)

        byE = np.concatenate([b_y[k * YW:(k + 1) * YW],
                              b_E[k * EW:(k + 1) * EW]])[:, None].astype(
            np.float32)                                    # [MW, 1]

        in_maps.append({
            "wg": wgk, "xt": xt, "hpt": hpt, "cpt": cptk, "bg": bgk,
            "wyE": wyE, "byE": byE,
        })
    return in_maps


def _get_nc():
    if "nc" not in _CACHE:
        _CACHE["nc"] = _build_nc()
    return _CACHE["nc"]


def _make_runner(nc):
    """Cached shard_map runner so repeat kernel() calls reuse the loaded
    executable (no re-jit / re-load; collective communicator stays warm)."""
    import jax
    from jax.sharding import Mesh, PartitionSpec
    from jax.experimental.shard_map import shard_map
    from concourse.bass2jax import (_bass_exec_p, partition_id_tensor,
                                    install_neuronx_cc_hook)
    install_neuronx_cc_hook()
    n_cores = NCORES
    partition_name = (nc.partition_id_tensor.name
                      if nc.partition_id_tensor else None)
    in_names, out_names, out_avals, zero_shapes = [], [], [], []
    for alloc in nc.m.functions[0].allocations:
        if not isinstance(alloc, mybir.MemoryLocationSet):
            continue
        name = alloc.memorylocations[0].name
        if alloc.kind == "ExternalInput":
            if name != partition_name:
                in_names.append(name)
        elif alloc.kind == "ExternalOutput":
            out_names.append(name)
            shape = tuple(alloc.tensor_shape)
            dtype = mybir.dt.np(alloc.dtype)
            out_avals.append(jax.core.ShapedArray(shape, dtype))
            zero_shapes.append((shape, dtype))
    n_params = len(in_names)
    n_outs = len(out_avals)
    all_in_names = list(in_names) + out_names
    if partition_name is not None:
        all_in_names.append(partition_name)
    donate = tuple(range(n_params, n_params + n_outs))

    def _body(*args):
        operands = list(args)
        if partition_name is not None:
            operands.append(partition_id_tensor())
        outs = _bass_exec_p.bind(
            *operands, out_avals=tuple(out_avals),
            in_names=tuple(all_in_names), out_names=tuple(out_names),
            lowering_input_output_aliases=(), sim_require_finite=True,
            sim_require_nnan=True, nc=nc)
        return tuple(outs)

    devices = jax.devices()[:n_cores]
    mesh = Mesh(np.asarray(devices), ("core",))
    in_specs = (PartitionSpec("core"),) * (n_params + n_outs)
    out_specs = (PartitionSpec("core"),) * n_outs
    sharded = jax.jit(
        shard_map(_body, mesh=mesh, in_specs=in_specs, out_specs=out_specs,
                  check_rep=False),
        donate_argnums=donate, keep_unused=True)

    def run(in_maps):
        per_core = [[np.asarray(m[name]) for name in in_names]
                    for m in in_maps]
        concat_in = [
            np.concatenate([per_core[c][i] for c in range(n_cores)], axis=0)
            for i in range(n_params)]
        concat_zeros = [
            np.zeros((n_cores * s[0], *s[1:]), dtp)
            for s, dtp in zero_shapes]
        out_arrs = sharded(*concat_in, *concat_zeros)
        return [
            {name: np.asarray(out_arrs[i]).reshape(
                n_cores, *zero_shapes[i][0])[c]
             for i, name in enumerate(out_names)}
            for c in range(n_cores)]
    return run


def _get_runner():
    if "run" not in _CACHE:
        _CACHE["run"] = _make_runner(_get_nc())
    return _CACHE["run"]


def kernel(**inputs):
    run = _get_runner()
    in_maps = _shard_inputs(inputs)
    results = run(in_maps)
    output = np.empty((B, P_VT), np.float32)
    interface = np.empty((B, P_ET), np.float32)
    for k in range(NCORES):
        oyE = results[k]["oyE"]
        output[:, k * YW:(k + 1) * YW] = oyE[:YW].T
        interface[:, k * EW:(k + 1) * EW] = oyE[YW:].T
    return (output, interface)


# revision 8
# speedup vs baseline: 1.0807x; 1.0807x over previous
"""Trainium2 Bass kernel for nn_Controller (4-layer LSTM-like controller + heads).

Sharding: tensor-parallel over the hidden dim H=1024 -> 128 h-columns per core
for the stacked gate GEMMs, with an AllGather of each layer's h to rebuild the
full hidden state for the next layer's recurrence and the output heads. Heads
are sharded by output column (64 of 512 y-cols + 32 of 256 E-cols per core).

The batch (512) is processed in two halves of 256 so each layer's AllGather
(bf16 [128,256] per core -> Mesh algorithm, ~6.5us) pipelines with the other
half's compute; the collective stream is the critical resource. All matmuls
run in bf16 with fp32 PSUM accumulation; the cell state c and the gate
nonlinearities stay in fp32.
"""

import numpy as np
import ml_dtypes

import concourse.bass as bass
import concourse.mybir as mybir
import concourse.tile as tile
from concourse import bacc
from concourse.bass_utils import run_bass_kernel_spmd

BF16 = ml_dtypes.bfloat16

# Problem constants (hardcoded; kernel.py must be self-contained).
P_X, P_H, P_L, P_VT, P_ET, B = 512, 1024, 4, 512, 256, 512
IN_DIM = P_X + 2 * P_H  # 2560
NCORES = 8
HS = P_H // NCORES       # 128 h-columns per core
YW = P_VT // NCORES      # 64 y output cols per core
EW = P_ET // NCORES      # 32 E output cols per core
MW = YW + EW             # 96 head output cols per core
G = 4                    # gates (i, f, o, s)
NK_X = P_X // 128        # 4 K-tiles from x
NK_H = P_H // 128        # 8 K-tiles from h
NKT = NK_X + 2 * NK_H    # 20 K-tiles total per layer
NPRE = NK_X + NK_H       # 12 K-tiles with no cross-core dependency
NK_HEAD = (P_L * P_H) // 128  # 32 K-tiles for the heads
HB = B // 2              # batch half

_CACHE = {}


def _build_nc():
    dt = mybir.dt
    AF = mybir.ActivationFunctionType

    nc = bacc.Bacc("TRN2", target_bir_lowering=False, debug=False,
                   num_devices=NCORES)

    wg_d = nc.dram_tensor("wg", [P_L, 128, NKT * G * HS], dt.bfloat16,
                          kind="ExternalInput")
    xt_d = nc.dram_tensor("xt", [128, NK_X * B], dt.bfloat16,
                          kind="ExternalInput")
    hpt_d = nc.dram_tensor("hpt", [P_L, 128, NK_H * B], dt.bfloat16,
                           kind="ExternalInput")
    cpt_d = nc.dram_tensor("cpt", [P_L, 128, B], dt.float32,
                           kind="ExternalInput")
    bg_d = nc.dram_tensor("bg", [128, P_L * G], dt.float32,
                          kind="ExternalInput")
    wyE_d = nc.dram_tensor("wyE", [128, NK_HEAD * MW], dt.bfloat16,
                           kind="ExternalInput")
    byE_d = nc.dram_tensor("byE", [MW, 1], dt.float32, kind="ExternalInput")
    out_d = nc.dram_tensor("oyE", [MW, B], dt.float32, kind="ExternalOutput")

    rg = [list(range(NCORES))]
    gate_order = [0, 3, 1, 2]  # i, s first (product needed first), then f, o

    with tile.TileContext(nc) as tc:
        with (
            tc.tile_pool(name="wpool", bufs=1) as wpool,
            tc.tile_pool(name="wgpool", bufs=2) as wgpool,
            tc.tile_pool(name="apool", bufs=2) as apool,
            tc.tile_pool(name="psum", bufs=1, space="PSUM") as psum,
            tc.tile_pool(name="dram", bufs=2, space="DRAM") as dram,
        ):
            # Tiny dummy AG issued first: absorbs the first-collective-op
            # overhead and the post-init doorbell gap while L0 computes.
            warm_i = dram.tile([8, 8], dt.bfloat16, name="warm_i", tag="wi")
            nc.gpsimd.dma_start(warm_i[:], xt_d[:8, :8])
            warm_o = dram.tile([NCORES * 8, 8], dt.bfloat16, name="warm_o",
                               tag="wo", addr_space="Shared")
            nc.gpsimd.collective_compute(
                "AllGather", mybir.AluOpType.bypass, replica_groups=rg,
                ins=[warm_i.opt()], outs=[warm_o.opt()])

            # Layer-0-critical loads first, in consumption order.
            xt_sb = wpool.tile([128, NK_X, B], dt.bfloat16, name="xt_sb")
            nc.sync.dma_start(xt_sb[:], xt_d.ap().rearrange(
                "p (k n) -> p k n", k=NK_X))
            wg_sb = [None] * P_L
            wg_sb[0] = wgpool.tile([128, NKT, G * HS], dt.bfloat16,
                                   name="wg0", tag="wg")
            wgv0 = wg_d[0].rearrange("p (k m) -> p k m", k=NKT)
            for c0, c1 in ((0, 4), (4, 8), (8, 12)):
                nc.sync.dma_start(wg_sb[0][:, c0:c1, :], wgv0[:, c0:c1, :])
            hpt_sb = [None] * P_L
            hpt_sb[0] = wpool.tile([128, NK_H, B], dt.bfloat16, name="hpt0",
                                   tag="hpt0")
            nc.sync.dma_start(hpt_sb[0][:], hpt_d[0].rearrange(
                "p (k n) -> p k n", k=NK_H))
            bg_sb = wpool.tile([128, P_L * G], dt.float32, name="bg_sb")
            nc.sync.dma_start(bg_sb[:], bg_d[:])
            cpt_sb = [None] * P_L
            cpt_sb[0] = wpool.tile([128, B], dt.float32, name="cpt0",
                                   tag="cpt0")
            nc.sync.dma_start(cpt_sb[0][:], cpt_d[0])

            # Remaining loads, layer-ordered so they stream just ahead of use.
            for l in range(1, P_L):
                hpt_sb[l] = wpool.tile([128, NK_H, B], dt.bfloat16,
                                       name=f"hpt{l}", tag=f"hpt{l}")
                nc.sync.dma_start(hpt_sb[l][:], hpt_d[l].rearrange(
                    "p (k n) -> p k n", k=NK_H))
                wg_sb[l] = wgpool.tile([128, NKT, G * HS], dt.bfloat16,
                                       name=f"wg{l}", tag="wg")
                wgv = wg_d[l].rearrange("p (k m) -> p k m", k=NKT)
                for c0, c1 in ((0, 5), (5, 10), (10, 15), (15, 20)):
                    nc.sync.dma_start(wg_sb[l][:, c0:c1, :], wgv[:, c0:c1, :])
                cpt_sb[l] = wpool.tile([128, B], dt.float32, name=f"cpt{l}",
                                       tag=f"cpt{l}")
                nc.sync.dma_start(cpt_sb[l][:], cpt_d[l])
            wyE_sb = wpool.tile([128, NK_HEAD, MW], dt.bfloat16, name="wyE_sb")
            nc.sync.dma_start(wyE_sb[:], wyE_d.ap().rearrange(
                "p (k m) -> p k m", k=NK_HEAD))
            byE_sb = wpool.tile([MW, 1], dt.float32, name="byE_sb")
            nc.sync.dma_start(byE_sb[:], byE_d[:])

            hfull = [None] * P_L  # gathered full h.T per layer (bf16)

            for l in range(P_L):
                nkt = NPRE if l == 0 else NKT

                def rhs(kt, hs):
                    if kt < NK_X:
                        return xt_sb[:, kt, hs]
                    if kt < NPRE:
                        return hpt_sb[l][:, kt - NK_X, hs]
                    return hfull[l - 1][:, kt - NPRE, hs]

                halves = [slice(0, HB), slice(HB, B)]
                ps = {}
                for X in (0, 1):
                    for g in gate_order:
                        ps[(g, X)] = psum.tile(
                            [128, HB], dt.float32, name=f"ps{l}{g}{X}",
                            tag=f"g{g}{X}")
                # Pre phase: contributions from x and h_prev (input-only deps).
                for X in (0, 1):
                    for g in gate_order:
                        for kt in range(NPRE):
                            nc.tensor.matmul(
                                ps[(g, X)][:],
                                wg_sb[l][:, kt, g * HS:(g + 1) * HS],
                                rhs(kt, halves[X]), start=(kt == 0),
                                stop=(kt == nkt - 1))
                # Prev phase per half (waits on that half's AllGather), then
                # activations + AllGather for that half.
                for X in (0, 1):
                    if l > 0:
                        for g in gate_order:
                            for kt in range(NPRE, nkt):
                                nc.tensor.matmul(
                                    ps[(g, X)][:],
                                    wg_sb[l][:, kt, g * HS:(g + 1) * HS],
                                    rhs(kt, halves[X]), start=False,
                                    stop=(kt == nkt - 1))
                    hs = halves[X]
                    bidx = lambda g: l * G + g
                    i_s = apool.tile([128, HB], dt.float32, name="i_s",
                                     tag="i_s")
                    nc.scalar.activation(i_s[:], ps[(0, X)][:], AF.Sigmoid,
                                         bias=bg_sb[:, bidx(0):bidx(0) + 1])
                    s_s = apool.tile([128, HB], dt.float32, name="s_s",
                                     tag="s_s")
                    nc.scalar.activation(s_s[:], ps[(3, X)][:], AF.Tanh,
                                         bias=bg_sb[:, bidx(3):bidx(3) + 1])
                    t1 = apool.tile([128, HB], dt.float32, name="t1", tag="t1")
                    nc.vector.tensor_mul(t1[:], i_s[:], s_s[:])
                    f_s = apool.tile([128, HB], dt.float32, name="f_s",
                                     tag="f_s")
                    nc.scalar.activation(f_s[:], ps[(1, X)][:], AF.Sigmoid,
                                         bias=bg_sb[:, bidx(1):bidx(1) + 1])
                    t2 = apool.tile([128, HB], dt.float32, name="t2", tag="t2")
                    nc.vector.tensor_mul(t2[:], f_s[:], cpt_sb[l][:, hs])
                    c_n = apool.tile([128, HB], dt.float32, name="c_n",
                                     tag="c_n")
                    nc.vector.tensor_add(c_n[:], t1[:], t2[:])
                    o_s = apool.tile([128, HB], dt.float32, name="o_s",
                                     tag="o_s")
                    nc.scalar.activation(o_s[:], ps[(2, X)][:], AF.Sigmoid,
                                         bias=bg_sb[:, bidx(2):bidx(2) + 1])
                    tc_n = apool.tile([128, HB], dt.float32, name="tc_n",
                                      tag="tc_n")
                    nc.scalar.activation(tc_n[:], c_n[:], AF.Tanh)
                    h_bf = apool.tile([128, HB], dt.bfloat16, name="h_bf",
                                      tag="h_bf")
                    nc.vector.tensor_mul(h_bf[:], o_s[:], tc_n[:])

                    ag_in = dram.tile([128, HB], dt.bfloat16,
                                      name=f"agin{l}{X}", tag="ag_in")
                    nc.gpsimd.dma_start(ag_in[:], h_bf[:])
                    ag_out = dram.tile([NCORES * 128, HB], dt.bfloat16,
                                       name=f"agout{l}{X}", tag="ag_out",
                                       addr_space="Shared")
                    nc.gpsimd.collective_compute(
                        "AllGather", mybir.AluOpType.bypass, replica_groups=rg,
                        ins=[ag_in.opt()], outs=[ag_out.opt()])
                    if hfull[l] is None:
                        hfull[l] = wpool.tile([128, NK_H, B], dt.bfloat16,
                                              name=f"hf{l}", tag=f"hf{l}")
                    agv = ag_out.rearrange("(r p) n -> p r n", p=128)
                    nc.sync.dma_start(hfull[l][:, 0:4, hs], agv[:, 0:4, :])
                    nc.sync.dma_start(hfull[l][:, 4:8, hs], agv[:, 4:8, :])

            # Output heads, per batch half: this core's [96, HB] slice of
            # (y | E), contracting over the full 4096-dim hidden state.
            for X in (0, 1):
                hs = slice(X * HB, (X + 1) * HB)
                ps_y = psum.tile([MW, HB], dt.float32, name=f"ps_y{X}",
                                 tag=f"g0{X}")
                for ki in range(NK_HEAD):
                    l, j = divmod(ki, NK_H)
                    nc.tensor.matmul(ps_y[:], wyE_sb[:, ki, :],
                                     hfull[l][:, j, hs],
                                     start=(ki == 0), stop=(ki == NK_HEAD - 1))
                outy = apool.tile([MW, HB], dt.float32, name="outy", tag="outy")
                nc.scalar.activation(outy[:], ps_y[:], AF.Identity,
                                     bias=byE_sb[:, 0:1])
                nc.sync.dma_start(out_d[:, hs], outy[:])

    nc.compile()
    return nc


def _shard_inputs(inputs):
    """Host-side: transpose/convert/shard the full inputs into 8 per-core maps."""
    x = np.ascontiguousarray(inputs["input_x"], dtype=np.float32)
    h_prev = np.ascontiguousarray(inputs["h_prev"], dtype=np.float32)
    c_prev = np.ascontiguousarray(inputs["c_prev"], dtype=np.float32)
    Wg = np.ascontiguousarray(inputs["Wg"], dtype=np.float32)
    bg = np.ascontiguousarray(inputs["bg"], dtype=np.float32)
    W_y = np.ascontiguousarray(inputs["W_y"], dtype=np.float32)
    b_y = np.ascontiguousarray(inputs["b_y"], dtype=np.float32)
    W_E = np.ascontiguousarray(inputs["W_E"], dtype=np.float32)
    b_E = np.ascontiguousarray(inputs["b_E"], dtype=np.float32)

    # Shared across cores.
    xt = x.T.reshape(NK_X, 128, B).transpose(1, 0, 2).reshape(128, NK_X * B)
    xt = xt.astype(BF16)
    hpt = h_prev.transpose(0, 2, 1).reshape(P_L, NK_H, 128, B)
    hpt = hpt.transpose(0, 2, 1, 3).reshape(P_L, 128, NK_H * B).astype(BF16)

    wg_t = Wg.transpose(0, 3, 1, 2)  # [L, IN_DIM, G, H]

    in_maps = []
    for k in range(NCORES):
        wgk = wg_t[:, :, :, k * HS:(k + 1) * HS]          # [L, IN_DIM, G, HS]
        wgk = wgk.reshape(P_L, NKT, 128, G * HS)           # (l, kt, p, m)
        wgk = wgk.transpose(0, 2, 1, 3)                    # (l, p, kt, m)
        wgk = np.ascontiguousarray(wgk).reshape(P_L, 128, NKT * G * HS)
        wgk = wgk.astype(BF16)

        bgk = bg[:, :, k * HS:(k + 1) * HS]                # [L, G, HS]
        bgk = np.ascontiguousarray(bgk.transpose(2, 0, 1)).reshape(
            128, P_L * G).astype(np.float32)

        cptk = np.ascontiguousarray(
            c_prev[:, :, k * HS:(k + 1) * HS].transpose(0, 2, 1)).astype(
            np.float32)                                    # [L, 128, B]

        wy = W_y[k * YW:(k + 1) * YW]                      # [YW, 4096]
        wE = W_E[k * EW:(k + 1) * EW]                      # [EW, 4096]
        wyE = np.concatenate([wy, wE], axis=0).T           # [4096, MW]
        wyE = wyE.reshape(NK_HEAD, 128, MW).transpose(1, 0, 2)
        wyE = np.ascontiguousarray(wyE).reshape(128, NK_HEAD * MW).astype(BF16)

        byE = np.concatenate([b_y[k * YW:(k + 1) * YW],
                              b_E[k * EW:(k + 1) * EW]])[:, None].astype(
            np.float32)                                    # [MW, 1]

        in_maps.append({
            "wg": wgk, "xt": xt, "hpt": hpt, "cpt": cptk, "bg": bgk,
            "wyE": wyE, "byE": byE,
        })
    return in_maps


def _get_nc():
    if "nc" not in _CACHE:
        _CACHE["nc"] = _build_nc()
    return _CACHE["nc"]


def _make_runner(nc):
    """Cached shard_map runner so repeat kernel() calls reuse the loaded
    executable (no re-jit / re-load; collective communicator stays warm)."""
    import jax
    from jax.sharding import Mesh, PartitionSpec
    from jax.experimental.shard_map import shard_map
    from concourse.bass2jax import (_bass_exec_p, partition_id_tensor,
                                    install_neuronx_cc_hook)
    install_neuronx_cc_hook()
    n_cores = NCORES
    partition_name = (nc.partition_id_tensor.name
                      if nc.partition_id_tensor else None)
    in_names, out_names, out_avals, zero_shapes = [], [], [], []
    for alloc in nc.m.functions[0].allocations:
        if not isinstance(alloc, mybir.MemoryLocationSet):
            continue
        name = alloc.memorylocations[0].name
        if alloc.kind == "ExternalInput":
            if name != partition_name:
                in_names.append(name)
        elif alloc.kind == "ExternalOutput":
            out_names.append(name)
            shape = tuple(alloc.tensor_shape)
            dtype = mybir.dt.np(alloc.dtype)
            out_avals.append(jax.core.ShapedArray(shape, dtype))
            zero_shapes.append((shape, dtype))
    n_params = len(in_names)
    n_outs = len(out_avals)
    all_in_names = list(in_names) + out_names
    if partition_name is not None:
        all_in_names.append(partition_name)
    donate = tuple(range(n_params, n_params + n_outs))

    def _body(*args):
        operands = list(args)
        if partition_name is not None:
            operands.append(partition_id_tensor())
        outs = _bass_exec_p.bind(
            *operands, out_avals=tuple(out_avals),
            in_names=tuple(all_in_names), out_names=tuple(out_names),
            lowering_input_output_aliases=(), sim_require_finite=True,
            sim_require_nnan=True, nc=nc)
        return tuple(outs)

    devices = jax.devices()[:n_cores]
    mesh = Mesh(np.asarray(devices), ("core",))
    in_specs = (PartitionSpec("core"),) * (n_params + n_outs)
    out_specs = (PartitionSpec("core"),) * n_outs
    sharded = jax.jit(
        shard_map(_body, mesh=mesh, in_specs=in_specs, out_specs=out_specs,
                  check_rep=False),
        donate_argnums=donate, keep_unused=True)

    def run(in_maps):
        per_core = [[np.asarray(m[name]) for name in in_names]
                    for m in in_maps]
        concat_in = [
            np.concatenate([per_core[c][i] for c in range(n_cores)], axis=0)
            for i in range(n_params)]
        concat_zeros = [
            np.zeros((n_cores * s[0], *s[1:]), dtp)
            for s, dtp in zero_shapes]
        out_arrs = sharded(*concat_in, *concat_zeros)
        return [
            {name: np.asarray(out_arrs[i]).reshape(
                n_cores, *zero_shapes[i][0])[c]
             for i, name in enumerate(out_names)}
            for c in range(n_cores)]
    return run


def _get_runner():
    if "run" not in _CACHE:
        _CACHE["run"] = _make_runner(_get_nc())
    return _CACHE["run"]


def kernel(**inputs):
    run = _get_runner()
    in_maps = _shard_inputs(inputs)
    results = run(in_maps)
    output = np.empty((B, P_VT), np.float32)
    interface = np.empty((B, P_ET), np.float32)
    for k in range(NCORES):
        oyE = results[k]["oyE"]
        output[:, k * YW:(k + 1) * YW] = oyE[:YW].T
        interface[:, k * EW:(k + 1) * EW] = oyE[YW:].T
    return (output, interface)
